# revision 22
# baseline (speedup 1.0000x reference)
"""CausalWanSelfAttention Trainium2 kernel (8-core SPMD, head-sharded).

Problem (hardcoded shapes): B=1, Lq=1560, H=12, D=128, CACHE=9360,
local_end_index=9360 -> attention window = full cache [0:9360), where
cache rows [7800:9360) are replaced by RoPE'd new keys / raw new values.

Sharding: 12 heads over 8 cores. Core c owns:
  - full head  hA = c            (all 1560 queries)
  - half head  hB = 8 + c//2     (queries [qoff, qoff+780), qoff=(c%2)*780)

Device algorithm per (head, query-range) task, flash-attention style with
S^T orientation (f32r matmuls = fp32-high single pass, k-chunks of 120,
query blocks of 390):
  K^T [d=128, 9360]: old-cache part DMA'd pre-transposed (host relayout),
  new part RoPE'd on device + PE-transposed. V kept natural [tok, d].
  S^T[chunk, qb] = (K^T chunk).T @ Q^T ; P^T = exp(S^T/sqrt(D)) on ACT
  O^T += (V chunk).T @ P^T chunk  accumulated in PSUM
  denom = ones.T @ (sum of P^T)   (two accumulator lanes: DVE 3:1 GPSIMD,
                                   folded by one PE matmul)
  out = (O^T).T * (1/denom)       (PE transpose + DVE scale), epilogue
                                  deferred one query-block for PE overlap
"""

import math
from contextlib import ExitStack

import numpy as np

import concourse.bass as bass
import concourse.bacc as bacc
import concourse.tile as tile
import concourse.mybir as mybir
from concourse import bass_utils
from concourse.masks import make_identity

F32 = mybir.dt.float32
F32R = mybir.dt.float32r
BF16 = mybir.dt.bfloat16
EXP = mybir.ActivationFunctionType.Exp

AV_BF16 = False  # P^T/V in bf16 for the AV pass (QK stays f32r); hurts accuracy ~7x

D = 128
LQ = 1560
H = 12
CACHE = 9360
OLD = 7800          # cache rows kept as-is
KC = 120            # k-chunk size (partition dim of S^T tiles)
NCH = CACHE // KC   # 78 chunks
SUB = 13            # chunks per K^T / V sub-tile
NSUB = NCH // SUB   # 6 sub-tiles
OLDSUB = OLD // (SUB * KC)  # 5 sub-tiles of old cache (pre-transposed)
QB = 390            # query block width
GRP = 3             # chunks per PSUM group / exp instruction
NGRP = NCH // GRP   # 26
SCALE = 1.0 / math.sqrt(D)
PIECES = [(0, 128), (128, 128), (256, 128), (384, QB - 384)]

_CACHED = {}


def _f(ap):
    return ap.bitcast(F32)


def _r(ap):
    return ap.bitcast(F32R)


def _rope_batch(nc, out, xst, cosst, sinst, te, to):
    """RoPE interleaved pairs, batched over chunks (f32r-rounded output)."""
    xp = _f(xst).rearrange("p c (d t) -> p c d t", t=2)
    op = out.rearrange("p c (d t) -> p c d t", t=2)
    xe, xo = xp[:, :, :, 0], xp[:, :, :, 1]
    oe, oo = op[:, :, :, 0], op[:, :, :, 1]
    mul = mybir.AluOpType.mult
    nc.vector.tensor_tensor(out=te, in0=xo, in1=sinst, op=mul)
    nc.vector.tensor_tensor(out=to, in0=xe, in1=sinst, op=mul)
    nc.vector.tensor_tensor(out=oe, in0=xe, in1=cosst, op=mul)
    nc.vector.tensor_tensor(out=oo, in0=xo, in1=cosst, op=mul)
    nc.vector.tensor_sub(oe, _f(oe), te)
    nc.vector.tensor_add(oo, _f(oo), to)


class _Ctx:
    pass


def _task_build(nc, E, tname, qd, knd, vnd, cktd, cvd):
    """Emit Q^T / K^T / V staging for one task. Returns (qt, kts, vsbs)."""
    # ---- Q^T: DMA natural, rope on DVE, PE-transpose into qt
    qt = E.qtpool.tile([128, LQ], F32R, tag="qt")
    qst = E.stpool.tile([KC, SUB, 128], F32R, tag="kst")
    rq = E.rppool.tile([KC, SUB, 128], F32R, tag="rq", bufs=1)
    te = E.rppool.tile([KC, SUB, 64], F32, tag="te", bufs=1)
    to = E.rppool.tile([KC, SUB, 64], F32, tag="to", bufs=1)
    if tname == "A":
        qch = [KC] * SUB
        nc.sync.dma_start(qst, _r(qd.rearrange("(c p) d -> p c d", p=KC)))
        _rope_batch(nc, rq, qst, E.cosk, E.sink, te, to)
    else:
        qch = [KC] * 6 + [60]
        nc.sync.dma_start(qst[:, 0:6], _r(qd[0:720].rearrange("(c p) d -> p c d", p=KC)))
        nc.sync.dma_start(qst[0:60, 6], _r(qd[720:780]))
        _rope_batch(nc, rq[:, 0:6], qst[:, 0:6], E.cosqB[:, 0:6], E.sinqB[:, 0:6],
                    te[:, 0:6], to[:, 0:6])
        _rope_batch(nc, rq[0:60, 6:7], qst[0:60, 6:7], E.cosqB[0:60, 6:7],
                    E.sinqB[0:60, 6:7], te[0:60, 6:7], to[0:60, 6:7])
    col = 0
    j = 0
    while j < len(qch):
        grp = qch[j:j + 4]
        tp = E.tpool.tile([128, 480], F32, tag="tp")
        off = 0
        for gi, w in enumerate(grp):
            nc.tensor.transpose(_r(tp[0:128, off:off + w]),
                                rq[0:w, j + gi, :], E.ident[0:w, 0:w])
            off += w
        nc.vector.tensor_copy(qt[:, col:col + off], _r(tp[:, 0:off]))
        col += off
        j += len(grp)

    # ---- K^T: sub-tiles 0..4 DMA'd pre-transposed; sub-tile 5 roped+transposed
    kts = []
    for s in range(OLDSUB):
        kt = E.ktpool.tile([128, SUB * KC], F32R, tag=f"kt{s}")
        kts.append(kt)
        nc.sync.dma_start(kt, _r(cktd[:, s * SUB * KC:(s + 1) * SUB * KC]))
    kt5 = E.ktpool.tile([128, SUB * KC], F32R, tag="kt5")
    kts.append(kt5)
    st = E.stpool.tile([KC, SUB, 128], F32R, tag="kst")
    nc.sync.dma_start(st, _r(knd.rearrange("(c p) d -> p c d", p=KC)))
    rk = E.rppool.tile([KC, SUB, 128], F32R, tag="rk", bufs=1)
    te = E.rppool.tile([KC, SUB, 64], F32, tag="te", bufs=1)
    to = E.rppool.tile([KC, SUB, 64], F32, tag="to", bufs=1)
    _rope_batch(nc, rk, st, E.cosk, E.sink, te, to)
    for g0 in range(0, SUB, 4):
        gn = min(4, SUB - g0)
        tp = E.tpool.tile([128, 480], F32, tag="tp")
        for gi in range(gn):
            nc.tensor.transpose(_r(tp[0:128, gi * KC:(gi + 1) * KC]),
                                rk[0:KC, g0 + gi, :], E.ident[0:KC, 0:KC])
        nc.vector.tensor_copy(kt5[:, g0 * KC:(g0 + gn) * KC], _r(tp[:, 0:gn * KC]))

    # ---- V: natural layout, 6 sub-tiles; bf16 mode casts via idle GPSIMD
    vsbs = []
    for s in range(NSUB):
        src = cvd[s * SUB * KC:(s + 1) * SUB * KC] if s < OLDSUB else vnd
        src = src.rearrange("(c p) d -> p c d", p=KC)
        if AV_BF16:
            vstg = E.stpool.tile([128, SUB * 128], F32, tag="vstg")
            nc.sync.dma_start(vstg[0:KC, :].rearrange("p (c d) -> p c d", d=128), src)
            vt = E.vpool.tile([128, SUB * 128], BF16, tag=f"vsb{s}")
            nc.gpsimd.tensor_copy(vt[0:KC], vstg[0:KC])
        else:
            vt = E.vpool.tile([128, SUB * 128], F32R, tag=f"vsb{s}")
            nc.sync.dma_start(vt[0:KC, :].rearrange("p (c d) -> p c d", d=128), _r(src))
        vsbs.append(vt)
    return qt, kts, vsbs


def _qb_main(nc, E, qt, kts, vsbs, qb):
    """Main sweep for one query block; returns state for the deferred epilogue."""
    qsl = qt[:, qb * QB:(qb + 1) * QB]
    opsum = E.opool.tile([128, QB], F32, tag="o")
    accD = E.accpool.tile([128, GRP, QB], F32, tag="accD")
    accG = E.accpool.tile([128, GRP, QB], F32, tag="accG")
    for g in range(NGRP):
        sg = E.spool.tile([128, GRP, 512], F32, tag="s")
        for c in range(GRP):
            ch = g * GRP + c
            kt = kts[ch // SUB]
            off = (ch % SUB) * KC
            nc.tensor.matmul(sg[0:KC, c, 0:QB], kt[:, off:off + KC],
                             qsl, start=True, stop=True)
        pg = E.ppool.tile([128, GRP, QB], BF16 if AV_BF16 else F32R, tag="p")
        nc.scalar.activation(pg[0:KC], sg[0:KC, :, 0:QB], EXP, scale=SCALE)
        pga = pg[0:KC] if AV_BF16 else _f(pg[0:KC])
        # two independent accumulator lanes: DVE (2 of 3) and GPSIMD (1 of 3)
        if g == 0:
            nc.vector.tensor_copy(accD[0:KC], pga)
        elif g == 1:
            nc.gpsimd.tensor_copy(accG[0:KC], pga)
        elif g % 3 == 1:
            nc.gpsimd.tensor_add(accG[0:KC], accG[0:KC], pga)
        else:
            nc.vector.tensor_add(accD[0:KC], accD[0:KC], pga)
        for c in range(GRP):
            ch = g * GRP + c
            vt = vsbs[ch // SUB]
            voff = (ch % SUB) * 128
            nc.tensor.matmul(opsum[:, 0:QB], vt[0:KC, voff:voff + 128],
                             pg[0:KC, c, 0:QB],
                             start=(ch == 0), stop=(ch == NCH - 1),
                             skip_group_check=True)
    # fold lanes: accf = sum over both lanes and 3 chunk-columns
    nc.gpsimd.tensor_add(accD[0:KC], accD[0:KC], accG[0:KC])
    nc.vector.tensor_add(accD[0:KC, 0], accD[0:KC, 0], accD[0:KC, 1])
    accf = E.accpool.tile([128, QB], F32R, tag="accf")
    nc.vector.tensor_add(accf[0:KC], accD[0:KC, 0], accD[0:KC, 2])
    # denominator row + O^T drain (frees opsum for the next query block)
    tpd = E.tpool.tile([128, 480], F32, tag="tp")
    nc.tensor.matmul(tpd[0:1, 0:QB], E.ones[0:KC, 0:1], accf[0:KC],
                     start=True, stop=True)
    dn = E.smallp.tile([1, QB], F32, tag="dn")
    nc.vector.tensor_copy(dn, tpd[0:1, 0:QB])
    otsb = E.smallp.tile([128, QB], F32R, tag="ot")
    nc.scalar.copy(otsb, opsum[:, 0:QB])
    return dn, otsb


def _qb_epilogue(nc, E, dn, otsb, od, qb):
    """Transpose + normalize + store one query block (runs one qb late)."""
    # all four 1/denom columns in one psum tile, one reciprocal
    tpr = E.tpool.tile([128, 480], F32, tag="tp")
    for i, (ps, pw) in enumerate(PIECES):
        nc.tensor.matmul(tpr[0:pw, 2 * i:2 * i + 1], dn[0:1, ps:ps + pw],
                         E.onesf[0:1, 0:1], start=True, stop=True)
    rt = E.smallp.tile([128, 8], F32, tag="rt")
    nc.vector.reciprocal(rt, tpr[0:128, 0:8])
    # all four O^T piece transposes into one psum tile
    tpo = E.tpool.tile([128, 4, 128], F32, tag="tp")
    for i, (ps, pw) in enumerate(PIECES):
        nc.tensor.transpose(_r(tpo[0:pw, i, :]), otsb[:, ps:ps + pw], E.ident)
    ofin = E.smallp.tile([128, 4, 128], F32, tag="of")
    for i, (ps, pw) in enumerate(PIECES):
        nc.vector.tensor_scalar_mul(ofin[0:pw, i, :], tpo[0:pw, i, :],
                                    rt[0:pw, 2 * i:2 * i + 1])
        nc.sync.dma_start(od[qb * QB + ps:qb * QB + ps + pw, :], ofin[0:pw, i, :])


def _emit(tc, ctx):
    nc = tc.nc
    f = {}
    for nm, shp in [
        ("qA", [LQ, D]), ("qB", [LQ // 2, D]),
        ("knA", [LQ, D]), ("knB", [LQ, D]),
        ("vnA", [LQ, D]), ("vnB", [LQ, D]),
        ("cktA", [D, OLD]), ("cktB", [D, OLD]),
        ("cvA", [OLD, D]), ("cvB", [OLD, D]),
        ("cosk", [LQ, D // 2]), ("sink", [LQ, D // 2]),
        ("cosqB", [LQ // 2, D // 2]), ("sinqB", [LQ // 2, D // 2]),
    ]:
        f[nm] = nc.dram_tensor(nm, shp, F32, kind="ExternalInput").ap()
    oA = nc.dram_tensor("oA", [LQ, D], F32, kind="ExternalOutput").ap()
    oB = nc.dram_tensor("oB", [LQ // 2, D], F32, kind="ExternalOutput").ap()

    E = _Ctx()
    E.cpool = ctx.enter_context(tc.tile_pool(name="const", bufs=1))
    E.stpool = ctx.enter_context(tc.tile_pool(name="stage", bufs=2))
    E.rppool = ctx.enter_context(tc.tile_pool(name="rope", bufs=2))
    E.ktpool = ctx.enter_context(tc.tile_pool(name="kt", bufs=1))
    E.vpool = ctx.enter_context(tc.tile_pool(name="vsb", bufs=1))
    E.qtpool = ctx.enter_context(tc.tile_pool(name="qt", bufs=2))
    E.ppool = ctx.enter_context(tc.tile_pool(name="pt", bufs=5))
    E.accpool = ctx.enter_context(tc.tile_pool(name="acc", bufs=2))
    E.smallp = ctx.enter_context(tc.tile_pool(name="small", bufs=2))
    E.spool = ctx.enter_context(tc.tile_pool(name="spsum", bufs=2, space="PSUM"))
    E.opool = ctx.enter_context(tc.tile_pool(name="opsum", bufs=1, space="PSUM"))
    E.tpool = ctx.enter_context(tc.tile_pool(name="tpsum", bufs=1, space="PSUM"))

    identf = E.cpool.tile([128, 128], F32, tag="identf")
    make_identity(nc, identf)
    E.ident = E.cpool.tile([128, 128], F32R, tag="ident")
    nc.vector.tensor_copy(E.ident[:], identf[:])
    E.onesf = E.cpool.tile([128, 1], F32, tag="onesf")
    nc.gpsimd.memset(E.onesf, 1.0)
    E.ones = E.cpool.tile([128, 1], F32R, tag="ones")
    nc.vector.tensor_copy(E.ones[:], E.onesf[:])

    E.cosk = E.cpool.tile([KC, SUB, 64], F32, tag="cosk")
    E.sink = E.cpool.tile([KC, SUB, 64], F32, tag="sink")
    nc.sync.dma_start(E.cosk, f["cosk"].rearrange("(c p) f -> p c f", p=KC))
    nc.sync.dma_start(E.sink, f["sink"].rearrange("(c p) f -> p c f", p=KC))
    E.cosqB = E.cpool.tile([KC, 7, 64], F32, tag="cosqB")
    E.sinqB = E.cpool.tile([KC, 7, 64], F32, tag="sinqB")
    nc.sync.dma_start(E.cosqB[:, 0:6], f["cosqB"][0:720].rearrange("(c p) f -> p c f", p=KC))
    nc.sync.dma_start(E.cosqB[0:60, 6], f["cosqB"][720:780])
    nc.sync.dma_start(E.sinqB[:, 0:6], f["sinqB"][0:720].rearrange("(c p) f -> p c f", p=KC))
    nc.sync.dma_start(E.sinqB[0:60, 6], f["sinqB"][720:780])

    tasks = [
        ("A", LQ, f["qA"], f["knA"], f["vnA"], f["cktA"], f["cvA"], oA),
        ("B", LQ // 2, f["qB"], f["knB"], f["vnB"], f["cktB"], f["cvB"], oB),
    ]

    # software-pipelined emission: builds early, epilogues one qb late
    pend = None  # (dn, otsb, od, qb)
    built = {}
    built["A"] = _task_build(nc, E, "A", *[tasks[0][i] for i in (2, 3, 4, 5, 6)])
    qbs = [("A", qb) for qb in range(LQ // QB)] + [("B", qb) for qb in range(LQ // 2 // QB)]
    for idx, (tname, qb) in enumerate(qbs):
        # emit task B's build two query blocks before task A ends
        if tname == "A" and qb == LQ // QB - 2:
            built["B"] = _task_build(nc, E, "B", *[tasks[1][i] for i in (2, 3, 4, 5, 6)])
        t = tasks[0] if tname == "A" else tasks[1]
        qt, kts, vsbs = built[tname]
        dn, otsb = _qb_main(nc, E, qt, kts, vsbs, qb)
        if pend is not None:
            _qb_epilogue(nc, E, *pend)
        pend = (dn, otsb, t[7], qb)
    _qb_epilogue(nc, E, *pend)


def _build():
    if "nc" in _CACHED:
        return _CACHED["nc"]
    nc = bacc.Bacc("TRN2", target_bir_lowering=False, debug=False,
                   enable_asserts=False, num_devices=8)
    with tile.TileContext(nc) as tc, ExitStack() as ctx:
        _emit(tc, ctx)
    nc.compile()
    _CACHED["nc"] = nc
    return nc


def _shard(q, k, v, cos, sin, cache_k, cache_v):
    """Build the 8 per-core input maps (slicing/relayout only)."""
    a = np.ascontiguousarray
    f32 = np.float32
    ins = []
    for c in range(8):
        hA = c
        hB = 8 + c // 2
        qoff = (c % 2) * (LQ // 2)
        ins.append({
            "qA": a(q[0, :, hA, :], dtype=f32),
            "qB": a(q[0, qoff:qoff + LQ // 2, hB, :], dtype=f32),
            "knA": a(k[0, :, hA, :], dtype=f32),
            "knB": a(k[0, :, hB, :], dtype=f32),
            "vnA": a(v[0, :, hA, :], dtype=f32),
            "vnB": a(v[0, :, hB, :], dtype=f32),
            "cktA": a(cache_k[0, 0:OLD, hA, :].T, dtype=f32),
            "cktB": a(cache_k[0, 0:OLD, hB, :].T, dtype=f32),
            "cvA": a(cache_v[0, 0:OLD, hA, :], dtype=f32),
            "cvB": a(cache_v[0, 0:OLD, hB, :], dtype=f32),
            "cosk": a(cos, dtype=f32),
            "sink": a(sin, dtype=f32),
            "cosqB": a(cos[qoff:qoff + LQ // 2], dtype=f32),
            "sinqB": a(sin[qoff:qoff + LQ // 2], dtype=f32),
        })
    return ins


def kernel(q, k, v, cos, sin, cache_k, cache_v, local_end_index, **run_kwargs):
    assert int(local_end_index) == CACHE, "kernel hardcodes local_end_index=9360"
    nc = _build()
    ins = _shard(np.asarray(q), np.asarray(k), np.asarray(v), np.asarray(cos),
                 np.asarray(sin), np.asarray(cache_k), np.asarray(cache_v))
    res = bass_utils.run_bass_kernel_spmd(nc, ins, core_ids=list(range(8)),
                                          **run_kwargs)
    out = np.empty((1, LQ, H, D), np.float32)
    for c in range(8):
        hA = c
        hB = 8 + c // 2
        qoff = (c % 2) * (LQ // 2)
        out[0, :, hA, :] = res.results[c]["oA"]
        out[0, qoff:qoff + LQ // 2, hB, :] = res.results[c]["oB"]
    if run_kwargs:
        kernel.last_result = res
    return out


# revision 24
# speedup vs baseline: 1.0886x; 1.0886x over previous
"""CausalWanSelfAttention Trainium2 kernel (8-core SPMD, head-sharded).

Problem (hardcoded shapes): B=1, Lq=1560, H=12, D=128, CACHE=9360,
local_end_index=9360 -> attention window = full cache [0:9360), where
cache rows [7800:9360) are replaced by RoPE'd new keys / raw new values.

Sharding: 12 heads over 8 cores. Core c owns:
  - full head  hA = c            (all 1560 queries)
  - half head  hB = 8 + c//2     (queries [qoff, qoff+780), qoff=(c%2)*780)

Device algorithm per (head, query-range) task, flash-attention style with
S^T orientation (f32r matmuls = fp32-high single pass, k-chunks of 120,
query blocks of 390):
  K^T [d=128, 9360]: old-cache part DMA'd pre-transposed (host relayout),
  new part RoPE'd on device + PE-transposed. V kept natural [tok, d].
  S^T[chunk, qb] = (K^T chunk).T @ Q^T ; P^T = exp(S^T/sqrt(D)) on ACT
  O^T += (V chunk).T @ P^T chunk  accumulated in PSUM
  denom = ones.T @ (sum of P^T)   (two accumulator lanes: DVE 3:1 GPSIMD,
                                   folded by one PE matmul)
  out = (O^T).T * (1/denom)       (PE transpose + DVE scale), epilogue
                                  deferred one query-block for PE overlap
"""

import math
from contextlib import ExitStack

import numpy as np

import concourse.bass as bass
import concourse.bacc as bacc
import concourse.tile as tile
import concourse.mybir as mybir
from concourse import bass_utils
from concourse.masks import make_identity

F32 = mybir.dt.float32
F32R = mybir.dt.float32r
BF16 = mybir.dt.bfloat16
EXP = mybir.ActivationFunctionType.Exp

AV_BF16 = False  # P^T/V in bf16 for the AV pass (QK stays f32r); hurts accuracy ~7x

D = 128
LQ = 1560
H = 12
CACHE = 9360
OLD = 7800          # cache rows kept as-is
KC = 120            # k-chunk size (partition dim of S^T tiles)
NCH = CACHE // KC   # 78 chunks
SUB = 13            # chunks per K^T / V sub-tile
NSUB = NCH // SUB   # 6 sub-tiles
OLDSUB = OLD // (SUB * KC)  # 5 sub-tiles of old cache (pre-transposed)
QB = 390            # query block width
GRP = 3             # chunks per PSUM group / exp instruction
NGRP = NCH // GRP   # 26
SCALE = 1.0 / math.sqrt(D)
PIECES = [(0, 128), (128, 128), (256, 128), (384, QB - 384)]

_CACHED = {}


def _f(ap):
    return ap.bitcast(F32)


def _r(ap):
    return ap.bitcast(F32R)


def _rope_batch(nc, out, xst, cosst, sinst, te, to):
    """RoPE interleaved pairs, batched over chunks (f32r-rounded output)."""
    xp = _f(xst).rearrange("p c (d t) -> p c d t", t=2)
    op = out.rearrange("p c (d t) -> p c d t", t=2)
    xe, xo = xp[:, :, :, 0], xp[:, :, :, 1]
    oe, oo = op[:, :, :, 0], op[:, :, :, 1]
    mul = mybir.AluOpType.mult
    nc.vector.tensor_tensor(out=te, in0=xo, in1=sinst, op=mul)
    nc.vector.tensor_tensor(out=to, in0=xe, in1=sinst, op=mul)
    nc.vector.tensor_tensor(out=oe, in0=xe, in1=cosst, op=mul)
    nc.vector.tensor_tensor(out=oo, in0=xo, in1=cosst, op=mul)
    nc.vector.tensor_sub(oe, _f(oe), te)
    nc.vector.tensor_add(oo, _f(oo), to)


class _Ctx:
    pass


def _task_build(nc, E, tname, qd, knd, vnd, cktd, cvd):
    """Emit Q^T / K^T / V staging for one task. Returns (qt, kts, vsbs)."""
    # ---- Q^T: DMA natural, rope on DVE, PE-transpose into qt
    qt = E.qtpool.tile([128, LQ], F32R, tag="qt")
    qst = E.stpool.tile([KC, SUB, 128], F32R, tag="kst")
    rq = E.rppool.tile([KC, SUB, 128], F32R, tag="rq", bufs=1)
    te = E.rppool.tile([KC, SUB, 64], F32, tag="te", bufs=1)
    to = E.rppool.tile([KC, SUB, 64], F32, tag="to", bufs=1)
    if tname == "A":
        qch = [KC] * SUB
        nc.sync.dma_start(qst, _r(qd.rearrange("(c p) d -> p c d", p=KC)))
        _rope_batch(nc, rq, qst, E.cosk, E.sink, te, to)
    else:
        qch = [KC] * 6 + [60]
        nc.sync.dma_start(qst[:, 0:6], _r(qd[0:720].rearrange("(c p) d -> p c d", p=KC)))
        nc.sync.dma_start(qst[0:60, 6], _r(qd[720:780]))
        _rope_batch(nc, rq[:, 0:6], qst[:, 0:6], E.cosqB[:, 0:6], E.sinqB[:, 0:6],
                    te[:, 0:6], to[:, 0:6])
        _rope_batch(nc, rq[0:60, 6:7], qst[0:60, 6:7], E.cosqB[0:60, 6:7],
                    E.sinqB[0:60, 6:7], te[0:60, 6:7], to[0:60, 6:7])
    col = 0
    j = 0
    while j < len(qch):
        grp = qch[j:j + 4]
        tp = E.tpool.tile([128, 480], F32, tag="tp")
        off = 0
        for gi, w in enumerate(grp):
            nc.tensor.transpose(_r(tp[0:128, off:off + w]),
                                rq[0:w, j + gi, :], E.ident[0:w, 0:w])
            off += w
        nc.vector.tensor_copy(qt[:, col:col + off], _r(tp[:, 0:off]))
        col += off
        j += len(grp)

    # ---- K^T: sub-tiles 0..4 DMA'd pre-transposed; sub-tile 5 roped+transposed
    kts = []
    for s in range(OLDSUB):
        kt = E.ktpool.tile([128, SUB * KC], F32R, tag=f"kt{s}")
        kts.append(kt)
        nc.sync.dma_start(kt, _r(cktd[:, s * SUB * KC:(s + 1) * SUB * KC]))
    kt5 = E.ktpool.tile([128, SUB * KC], F32R, tag="kt5")
    kts.append(kt5)
    st = E.stpool.tile([KC, SUB, 128], F32R, tag="kst")
    nc.sync.dma_start(st, _r(knd.rearrange("(c p) d -> p c d", p=KC)))
    rk = E.rppool.tile([KC, SUB, 128], F32R, tag="rk", bufs=1)
    te = E.rppool.tile([KC, SUB, 64], F32, tag="te", bufs=1)
    to = E.rppool.tile([KC, SUB, 64], F32, tag="to", bufs=1)
    _rope_batch(nc, rk, st, E.cosk, E.sink, te, to)
    for g0 in range(0, SUB, 4):
        gn = min(4, SUB - g0)
        tp = E.tpool.tile([128, 480], F32, tag="tp")
        for gi in range(gn):
            nc.tensor.transpose(_r(tp[0:128, gi * KC:(gi + 1) * KC]),
                                rk[0:KC, g0 + gi, :], E.ident[0:KC, 0:KC])
        nc.vector.tensor_copy(kt5[:, g0 * KC:(g0 + gn) * KC], _r(tp[:, 0:gn * KC]))

    # ---- V: natural layout, 6 sub-tiles; bf16 mode casts via idle GPSIMD
    vsbs = []
    for s in range(NSUB):
        src = cvd[s * SUB * KC:(s + 1) * SUB * KC] if s < OLDSUB else vnd
        src = src.rearrange("(c p) d -> p c d", p=KC)
        if AV_BF16:
            vstg = E.stpool.tile([128, SUB * 128], F32, tag="vstg")
            nc.sync.dma_start(vstg[0:KC, :].rearrange("p (c d) -> p c d", d=128), src)
            vt = E.vpool.tile([128, SUB * 128], BF16, tag=f"vsb{s}")
            nc.gpsimd.tensor_copy(vt[0:KC], vstg[0:KC])
        else:
            vt = E.vpool.tile([128, SUB * 128], F32R, tag=f"vsb{s}")
            nc.sync.dma_start(vt[0:KC, :].rearrange("p (c d) -> p c d", d=128), _r(src))
        vsbs.append(vt)
    return qt, kts, vsbs


def _qb_main(nc, E, qt, kts, vsbs, qb):
    """Main sweep for one query block; returns state for the deferred epilogue."""
    qsl = qt[:, qb * QB:(qb + 1) * QB]
    opsum = E.opool.tile([128, QB], F32, tag="o")
    accD = E.accpool.tile([128, GRP, QB], F32, tag="accD")
    accG = E.accpool.tile([128, GRP, QB], F32, tag="accG")
    for g in range(NGRP):
        sg = E.spool.tile([128, GRP, 512], F32, tag="s")
        for c in range(GRP):
            ch = g * GRP + c
            kt = kts[ch // SUB]
            off = (ch % SUB) * KC
            nc.tensor.matmul(sg[0:KC, c, 0:QB], kt[:, off:off + KC],
                             qsl, start=True, stop=True)
        pg = E.ppool.tile([128, GRP, QB], BF16 if AV_BF16 else F32R, tag="p")
        nc.scalar.activation(pg[0:KC], sg[0:KC, :, 0:QB], EXP, scale=SCALE)
        pga = pg[0:KC] if AV_BF16 else _f(pg[0:KC])
        # two independent accumulator lanes: DVE (2 of 3) and GPSIMD (1 of 3)
        if g == 0:
            nc.vector.tensor_copy(accD[0:KC], pga)
        elif g == 1:
            nc.gpsimd.tensor_copy(accG[0:KC], pga)
        elif g % 4 == 1:
            nc.gpsimd.tensor_add(accG[0:KC], accG[0:KC], pga)
        else:
            nc.vector.tensor_add(accD[0:KC], accD[0:KC], pga)
        for c in range(GRP):
            ch = g * GRP + c
            vt = vsbs[ch // SUB]
            voff = (ch % SUB) * 128
            nc.tensor.matmul(opsum[:, 0:QB], vt[0:KC, voff:voff + 128],
                             pg[0:KC, c, 0:QB],
                             start=(ch == 0), stop=(ch == NCH - 1),
                             skip_group_check=True)
    # fold lanes: accf = sum over both lanes and 3 chunk-columns
    nc.vector.tensor_add(accD[0:KC], accD[0:KC], accG[0:KC])
    nc.vector.tensor_add(accD[0:KC, 0], accD[0:KC, 0], accD[0:KC, 1])
    accf = E.accpool.tile([128, QB], F32R, tag="accf")
    nc.vector.tensor_add(accf[0:KC], accD[0:KC, 0], accD[0:KC, 2])
    # denominator row + O^T drain (frees opsum for the next query block)
    tpd = E.tpool.tile([128, 480], F32, tag="tp")
    nc.tensor.matmul(tpd[0:1, 0:QB], E.ones[0:KC, 0:1], accf[0:KC],
                     start=True, stop=True)
    dn = E.smallp.tile([1, QB], F32, tag="dn")
    nc.vector.tensor_copy(dn, tpd[0:1, 0:QB])
    otsb = E.smallp.tile([128, QB], F32R, tag="ot")
    nc.scalar.copy(otsb, opsum[:, 0:QB])
    return dn, otsb


def _qb_epilogue(nc, E, dn, otsb, od, qb):
    """Transpose + normalize + store one query block (runs one qb late)."""
    # all four 1/denom columns in one psum tile, one reciprocal
    tpr = E.tpool.tile([128, 480], F32, tag="tp")
    for i, (ps, pw) in enumerate(PIECES):
        nc.tensor.matmul(tpr[0:pw, 2 * i:2 * i + 1], dn[0:1, ps:ps + pw],
                         E.onesf[0:1, 0:1], start=True, stop=True)
    rt = E.smallp.tile([128, 8], F32, tag="rt")
    nc.vector.reciprocal(rt, tpr[0:128, 0:8])
    # all four O^T piece transposes into one psum tile
    tpo = E.tpool.tile([128, 4, 128], F32, tag="tp")
    for i, (ps, pw) in enumerate(PIECES):
        nc.tensor.transpose(_r(tpo[0:pw, i, :]), otsb[:, ps:ps + pw], E.ident)
    ofin = E.smallp.tile([128, 4, 128], F32, tag="of")
    for i, (ps, pw) in enumerate(PIECES):
        nc.vector.tensor_scalar_mul(ofin[0:pw, i, :], tpo[0:pw, i, :],
                                    rt[0:pw, 2 * i:2 * i + 1])
        nc.sync.dma_start(od[qb * QB + ps:qb * QB + ps + pw, :], ofin[0:pw, i, :])


def _emit(tc, ctx):
    nc = tc.nc
    f = {}
    for nm, shp in [
        ("qA", [LQ, D]), ("qB", [LQ // 2, D]),
        ("knA", [LQ, D]), ("knB", [LQ, D]),
        ("vnA", [LQ, D]), ("vnB", [LQ, D]),
        ("cktA", [D, OLD]), ("cktB", [D, OLD]),
        ("cvA", [OLD, D]), ("cvB", [OLD, D]),
        ("cosk", [LQ, D // 2]), ("sink", [LQ, D // 2]),
        ("cosqB", [LQ // 2, D // 2]), ("sinqB", [LQ // 2, D // 2]),
    ]:
        f[nm] = nc.dram_tensor(nm, shp, F32, kind="ExternalInput").ap()
    oA = nc.dram_tensor("oA", [LQ, D], F32, kind="ExternalOutput").ap()
    oB = nc.dram_tensor("oB", [LQ // 2, D], F32, kind="ExternalOutput").ap()

    E = _Ctx()
    E.cpool = ctx.enter_context(tc.tile_pool(name="const", bufs=1))
    E.stpool = ctx.enter_context(tc.tile_pool(name="stage", bufs=2))
    E.rppool = ctx.enter_context(tc.tile_pool(name="rope", bufs=2))
    E.ktpool = ctx.enter_context(tc.tile_pool(name="kt", bufs=1))
    E.vpool = ctx.enter_context(tc.tile_pool(name="vsb", bufs=1))
    E.qtpool = ctx.enter_context(tc.tile_pool(name="qt", bufs=2))
    E.ppool = ctx.enter_context(tc.tile_pool(name="pt", bufs=5))
    E.accpool = ctx.enter_context(tc.tile_pool(name="acc", bufs=2))
    E.smallp = ctx.enter_context(tc.tile_pool(name="small", bufs=2))
    E.spool = ctx.enter_context(tc.tile_pool(name="spsum", bufs=2, space="PSUM"))
    E.opool = ctx.enter_context(tc.tile_pool(name="opsum", bufs=1, space="PSUM"))
    E.tpool = ctx.enter_context(tc.tile_pool(name="tpsum", bufs=1, space="PSUM"))

    identf = E.cpool.tile([128, 128], F32, tag="identf")
    make_identity(nc, identf)
    E.ident = E.cpool.tile([128, 128], F32R, tag="ident")
    nc.vector.tensor_copy(E.ident[:], identf[:])
    E.onesf = E.cpool.tile([128, 1], F32, tag="onesf")
    nc.gpsimd.memset(E.onesf, 1.0)
    E.ones = E.cpool.tile([128, 1], F32R, tag="ones")
    nc.vector.tensor_copy(E.ones[:], E.onesf[:])

    E.cosk = E.cpool.tile([KC, SUB, 64], F32, tag="cosk")
    E.sink = E.cpool.tile([KC, SUB, 64], F32, tag="sink")
    nc.sync.dma_start(E.cosk, f["cosk"].rearrange("(c p) f -> p c f", p=KC))
    nc.sync.dma_start(E.sink, f["sink"].rearrange("(c p) f -> p c f", p=KC))
    E.cosqB = E.cpool.tile([KC, 7, 64], F32, tag="cosqB")
    E.sinqB = E.cpool.tile([KC, 7, 64], F32, tag="sinqB")
    nc.sync.dma_start(E.cosqB[:, 0:6], f["cosqB"][0:720].rearrange("(c p) f -> p c f", p=KC))
    nc.sync.dma_start(E.cosqB[0:60, 6], f["cosqB"][720:780])
    nc.sync.dma_start(E.sinqB[:, 0:6], f["sinqB"][0:720].rearrange("(c p) f -> p c f", p=KC))
    nc.sync.dma_start(E.sinqB[0:60, 6], f["sinqB"][720:780])

    tasks = [
        ("A", LQ, f["qA"], f["knA"], f["vnA"], f["cktA"], f["cvA"], oA),
        ("B", LQ // 2, f["qB"], f["knB"], f["vnB"], f["cktB"], f["cvB"], oB),
    ]

    # software-pipelined emission: builds early, epilogues one qb late
    pend = None  # (dn, otsb, od, qb)
    built = {}
    built["A"] = _task_build(nc, E, "A", *[tasks[0][i] for i in (2, 3, 4, 5, 6)])
    qbs = [("A", qb) for qb in range(LQ // QB)] + [("B", qb) for qb in range(LQ // 2 // QB)]
    for idx, (tname, qb) in enumerate(qbs):
        # emit task B's build two query blocks before task A ends
        if tname == "A" and qb == LQ // QB - 2:
            built["B"] = _task_build(nc, E, "B", *[tasks[1][i] for i in (2, 3, 4, 5, 6)])
        t = tasks[0] if tname == "A" else tasks[1]
        qt, kts, vsbs = built[tname]
        dn, otsb = _qb_main(nc, E, qt, kts, vsbs, qb)
        if pend is not None:
            _qb_epilogue(nc, E, *pend)
        pend = (dn, otsb, t[7], qb)
    _qb_epilogue(nc, E, *pend)


def _build():
    if "nc" in _CACHED:
        return _CACHED["nc"]
    nc = bacc.Bacc("TRN2", target_bir_lowering=False, debug=False,
                   enable_asserts=False, num_devices=8)
    with tile.TileContext(nc) as tc, ExitStack() as ctx:
        _emit(tc, ctx)
    nc.compile()
    _CACHED["nc"] = nc
    return nc


def _shard(q, k, v, cos, sin, cache_k, cache_v):
    """Build the 8 per-core input maps (slicing/relayout only)."""
    a = np.ascontiguousarray
    f32 = np.float32
    ins = []
    for c in range(8):
        hA = c
        hB = 8 + c // 2
        qoff = (c % 2) * (LQ // 2)
        ins.append({
            "qA": a(q[0, :, hA, :], dtype=f32),
            "qB": a(q[0, qoff:qoff + LQ // 2, hB, :], dtype=f32),
            "knA": a(k[0, :, hA, :], dtype=f32),
            "knB": a(k[0, :, hB, :], dtype=f32),
            "vnA": a(v[0, :, hA, :], dtype=f32),
            "vnB": a(v[0, :, hB, :], dtype=f32),
            "cktA": a(cache_k[0, 0:OLD, hA, :].T, dtype=f32),
            "cktB": a(cache_k[0, 0:OLD, hB, :].T, dtype=f32),
            "cvA": a(cache_v[0, 0:OLD, hA, :], dtype=f32),
            "cvB": a(cache_v[0, 0:OLD, hB, :], dtype=f32),
            "cosk": a(cos, dtype=f32),
            "sink": a(sin, dtype=f32),
            "cosqB": a(cos[qoff:qoff + LQ // 2], dtype=f32),
            "sinqB": a(sin[qoff:qoff + LQ // 2], dtype=f32),
        })
    return ins


def kernel(q, k, v, cos, sin, cache_k, cache_v, local_end_index, **run_kwargs):
    assert int(local_end_index) == CACHE, "kernel hardcodes local_end_index=9360"
    nc = _build()
    ins = _shard(np.asarray(q), np.asarray(k), np.asarray(v), np.asarray(cos),
                 np.asarray(sin), np.asarray(cache_k), np.asarray(cache_v))
    res = bass_utils.run_bass_kernel_spmd(nc, ins, core_ids=list(range(8)),
                                          **run_kwargs)
    out = np.empty((1, LQ, H, D), np.float32)
    for c in range(8):
        hA = c
        hB = 8 + c // 2
        qoff = (c % 2) * (LQ // 2)
        out[0, :, hA, :] = res.results[c]["oA"]
        out[0, qoff:qoff + LQ // 2, hB, :] = res.results[c]["oB"]
    if run_kwargs:
        kernel.last_result = res
    return out
